# revision 21
# baseline (speedup 1.0000x reference)
"""AttnBlock (GroupNorm + single-head LxL attention + residual) on 8 trn2 cores.

Data-parallel over batch: core b handles sample b (full 2048x2048 attention).
All large matmuls run as fp8 e4m3 with perf_mode=DoubleRow: operands are 3D
[K=128, 2, free] access patterns contracting an effective K=256 per
instruction at ~0.5 cycles/output-column (2x the fp32r/bf16 stream rate).

Layout strategy (per core):
  x                  : 4 f32 tiles [128, 2048] (stats read these)
  H, Q, K, V         : one fp8 tile each [128, 4, 2048]; dim1 = ci tile index,
                       DoubleRow pair p uses slice [:, 2p:2p+2, :]
  weights            : host-prescaled (x16) transposed fp8 [128, 4, 512]
                       (dim1 = ci, free = out channel); evacs scale by 1/16
  S^T = K^T Q        : [j, i] psum tiles; exp() with bias -2 (cancels in the
                       softmax normalization; keeps e4m3 under its 240 max)
                       writes fp8 P~^T into pt8 [128, 16, 512] (dim1 = jt)
  W~^T = (Wo V)^T    : fp8 [128, 16, 512] (dim1 = jt, free = out channel)
  out^T[i, o]        : psum accum over 8 jt-pairs; row sums of P~ accumulate
                       as fp8 ones-matmuls [1, 512]; final evac does
                       out = psum * (1/rowsum) + (x^T + bo) as in the f32r
                       version (DMA-scatter transpose mid-block, K=1 matmul
                       transpose on the last block).
  GroupNorm rsqrt uses exp(-0.5*ln(var+eps)) so the ACT engine stays in the
  natural_log_exp table set for the whole kernel (no mid-kernel table switch).
"""

import numpy as np

C = 512
L = 2048
G = 32
GS = C // G          # 16 channels per group
EPS = 1e-6
CT = C // 128        # 4 channel tiles
JT = L // 128        # 16 j tiles
NB = 512             # matmul moving free dim / chunk size
LB = L // NB         # 4 i-blocks
NCORES = 8
WS = 16.0            # weight prescale before fp8 quantization
EB = -2.0            # exp bias (uniform factor e^-2, cancels in softmax)

_CACHE = {}


def _build():
    import concourse.bacc as bacc
    import concourse.tile as tile
    from concourse import mybir
    from concourse.alu_op_type import AluOpType
    from contextlib import ExitStack

    F32 = mybir.dt.float32
    F8 = mybir.dt.float8e4
    AF = mybir.ActivationFunctionType
    AX = mybir.AxisListType
    DR = mybir.MatmulPerfMode.DoubleRow

    nc = bacc.Bacc("TRN2", target_bir_lowering=False, debug=False, num_devices=NCORES)

    _ctr = [0]

    def nm(base):
        _ctr[0] += 1
        return f"{base}_{_ctr[0]}"

    x_d = nc.declare_dram_parameter("x8", [128, CT * L], F8, isOutput=False)
    xt_d = nc.declare_dram_parameter("xt", [L, C], F32, isOutput=False)
    wq_d = nc.declare_dram_parameter("wq8", [128, CT * C], F8, isOutput=False)
    wk_d = nc.declare_dram_parameter("wk8", [128, CT * C], F8, isOutput=False)
    wvo_d = nc.declare_dram_parameter("wvo8", [128, CT * C], F8, isOutput=False)
    cvec_d = nc.declare_dram_parameter("cvec", [128, 5 * CT], F32, isOutput=False)
    gmil_d = nc.declare_dram_parameter("gmil", [128, G * CT], F32, isOutput=False)
    gmT_d = nc.declare_dram_parameter("gmT", [G, C], F32, isOutput=False)
    one8_d = nc.declare_dram_parameter("one8", [128, 32], F8, isOutput=False)
    yt_d = nc.declare_dram_parameter("yt", [L, C], F32, isOutput=True)

    scale = float(1.0 / np.sqrt(C))

    with tile.TileContext(nc) as tc, ExitStack() as ctx:
        consts = ctx.enter_context(tc.tile_pool(name="consts", bufs=1))
        small = ctx.enter_context(tc.tile_pool(name="small", bufs=4))
        xin_p = ctx.enter_context(tc.tile_pool(name="scr", bufs=2))
        x_p = ctx.enter_context(tc.tile_pool(name="xp", bufs=4))
        h_p = ctx.enter_context(tc.tile_pool(name="hp", bufs=1))
        q_p = ctx.enter_context(tc.tile_pool(name="qp", bufs=1))
        k_p = ctx.enter_context(tc.tile_pool(name="kp", bufs=1))
        v_p = ctx.enter_context(tc.tile_pool(name="vp", bufs=1))
        w_p = ctx.enter_context(tc.tile_pool(name="wp", bufs=4))
        wt_p = ctx.enter_context(tc.tile_pool(name="wtp", bufs=1))
        p_p = ctx.enter_context(tc.tile_pool(name="pp", bufs=2))
        io_p = ctx.enter_context(tc.tile_pool(name="io", bufs=4))
        xt_p = ctx.enter_context(tc.tile_pool(name="xtp", bufs=5))
        ps_mm = ctx.enter_context(tc.tile_pool(name="psmm", bufs=5, space="PSUM"))
        ps_s = ctx.enter_context(tc.tile_pool(name="pss", bufs=3, space="PSUM"))

        gmil_sb = consts.tile([128, G * CT], F32, name=nm("gmil"), tag="gmil")
        nc.sync.dma_start(out=gmil_sb[:], in_=gmil_d[:, :])
        # x (fp8) next on the queue: stats (and everything after) gate on the
        # full x arrival, so it precedes the other small loads; separate
        # tiles per ct so each stat reduction fires as its chunk lands
        x8_t = []
        for ct in range(CT):
            x8 = x_p.tile([128, L], F8, name=nm("x8"), tag="x8")
            x8_t.append(x8)
            nc.sync.dma_start(out=x8[:], in_=x_d[:, ct * L:(ct + 1) * L])
        one8_t = consts.tile([128, 2, 16], F8, name=nm("one8"), tag="one8")
        nc.sync.dma_start(out=one8_t[:], in_=one8_d[:, :])
        onesf = consts.tile([1, 1], F32, name=nm("onesf"), tag="onesf")
        nc.vector.memset(onesf[:], 1.0)
        eps_t = consts.tile([G, 1], F32, name=nm("eps"), tag="eps")
        nc.vector.memset(eps_t[:], EPS)
        inv16 = consts.tile([128, 1], F32, name=nm("inv16"), tag="inv16")
        nc.vector.memset(inv16[:], 1.0 / WS)
        ebias = consts.tile([128, 1], F32, name=nm("ebias"), tag="ebias")
        nc.vector.memset(ebias[:], EB)
        # warm-up matmuls on the early-arriving mask tile (f32): keep the PE
        # at full clock and busy while x streams in and stats run
        for i in range(22):
            wps = ps_mm.tile([128, 128], F32, name=nm("warm"), tag="mm")
            nc.tensor.matmul(wps[:], gmil_sb[:, 0:128], gmil_sb[:, 0:128],
                             start=True, stop=True)
        cv_sb = consts.tile([128, 5 * CT], F32, name=nm("cv"), tag="cv")
        nc.sync.dma_start(out=cv_sb[:], in_=cvec_d[:, :])
        gmT_sb = consts.tile([G, C], F32, name=nm("gmT"), tag="gmT")
        nc.sync.dma_start(out=gmT_sb[:], in_=gmT_d[:, :])

        gm_sb = [gmil_sb[:, ct * G:(ct + 1) * G] for ct in range(CT)]
        bq_t = [cv_sb[:, ct * 5 + 0:ct * 5 + 1] for ct in range(CT)]
        bk_t = [cv_sb[:, ct * 5 + 1:ct * 5 + 2] for ct in range(CT)]
        bv_t = [cv_sb[:, ct * 5 + 2:ct * 5 + 3] for ct in range(CT)]
        gnw_t = [cv_sb[:, ct * 5 + 3:ct * 5 + 4] for ct in range(CT)]
        gnb_t = [cv_sb[:, ct * 5 + 4:ct * 5 + 5] for ct in range(CT)]

        def load_w8(w_dram):
            w = w_p.tile([128, CT, C], F8, name=nm("w"), tag="w")
            nc.sync.dma_start(out=w[:], in_=w_dram[:, :])
            return w

        def conv(bias_t, dst, wsb, jt_layout=False, act_evac=False):
            # dst: fp8 4D; DoubleRow contracts ci pairs; all pair strides
            # small so the fp8 stream runs at the 0.5 cyc/col rate
            for co in range(CT):
                pss = [ps_mm.tile([128, NB], F32, name=nm("mm"), tag="mm")
                       for _ in range(L // NB)]
                for p in range(2):
                    for lc in range(L // NB):
                        nc.tensor.matmul(
                            pss[lc][:],
                            wsb[:, 2 * p:2 * p + 2, co * 128:(co + 1) * 128],
                            h8[:, lc, 2 * p:2 * p + 2, :],
                            start=(p == 0), stop=(p == 1), perf_mode=DR)
                for lc in range(L // NB):
                    if jt_layout:
                        o = dst[:, 4 * lc:4 * lc + 4, co, :]
                    else:
                        o = dst[:, lc, co, :]
                    if act_evac:
                        nc.scalar.activation(out=o, in_=pss[lc][:],
                                             func=AF.Identity, bias=bias_t[co],
                                             scale=1.0 / WS)
                    else:
                        nc.vector.tensor_scalar(
                            out=o, in0=pss[lc][:],
                            scalar1=inv16[:], scalar2=bias_t[co],
                            op0=AluOpType.mult, op1=AluOpType.add)

        # ---- GroupNorm pass 1: per-channel sum and sum-of-squares ----
        stats = []
        for ct in range(CT):
            st = small.tile([128, 2], F32, name=nm("st"), tag=f"st{ct}")
            stats.append(st)
            nc.vector.reduce_sum(out=st[:, 0:1], in_=x8_t[ct][:], axis=AX.X)
            scr = xin_p.tile([128, L], F32, name=nm("scr"), tag="scr")
            nc.scalar.activation(out=scr[:], in_=x8_t[ct][:], func=AF.Square,
                                 accum_out=st[:, 1:2])

        # group-reduce the per-channel stats: [32, 2] = sum over channels in group
        gps = ps_s.tile([G, 2], F32, name=nm("s"), tag="s")
        for ct in range(CT):
            nc.tensor.matmul(gps[:], gm_sb[ct], stats[ct][:],
                             start=(ct == 0), stop=(ct == CT - 1))
        gmv = small.tile([G, 2], F32, name=nm("gmv"), tag="gmv")
        nc.scalar.mul(out=gmv[:], in_=gps[:], mul=1.0 / (GS * L))
        msq = small.tile([G, 1], F32, name=nm("msq"), tag="msq")
        nc.vector.tensor_mul(out=msq[:], in0=gmv[:, 0:1], in1=gmv[:, 0:1])
        var = small.tile([G, 1], F32, name=nm("var"), tag="var")
        nc.vector.tensor_sub(out=var[:], in0=gmv[:, 1:2], in1=msq[:])
        # 1/sqrt(var+eps) = exp(-0.5*ln(var+eps)): stays in the exp table set
        lnv = small.tile([G, 1], F32, name=nm("lnv"), tag="lnv")
        nc.scalar.activation(out=lnv[:], in_=var[:], func=AF.Ln,
                             bias=eps_t[:], scale=1.0)
        mr = small.tile([G, 2], F32, name=nm("mr"), tag="mr")
        nc.vector.tensor_copy(out=mr[:, 0:1], in_=gmv[:, 0:1])
        nc.scalar.activation(out=mr[:, 1:2], in_=lnv[:], func=AF.Exp,
                             scale=-0.5)

        # broadcast group mean/rstd back to channels, fold in gn weight/bias
        s_t, t_t = [], []
        for ct in range(CT):
            bps = ps_s.tile([128, 2], F32, name=nm("s"), tag="s")
            nc.tensor.matmul(bps[:], gmT_sb[:, ct * 128:(ct + 1) * 128], mr[:],
                             start=True, stop=True)
            s_ = small.tile([128, 1], F32, name=nm("sc"), tag=f"sc{ct}")
            nc.vector.tensor_mul(out=s_[:], in0=bps[:, 1:2], in1=gnw_t[ct])
            tmp = small.tile([128, 1], F32, name=nm("tmp"), tag="tmp")
            nc.vector.tensor_mul(out=tmp[:], in0=bps[:, 0:1], in1=s_[:])
            t_ = small.tile([128, 1], F32, name=nm("tc"), tag=f"tc{ct}")
            nc.vector.tensor_sub(out=t_[:], in0=gnb_t[ct], in1=tmp[:])
            s_t.append(s_)
            t_t.append(t_)

        # ---- GroupNorm pass 2: H = s*x + t -> fp8, in conv consumption
        # order (lc chunks couple all four ct tiles via the ci pairing) ----
        h8 = h_p.tile([128, L // NB, CT, NB], F8, name=nm("h8"), tag="h8")
        for lc in range(L // NB):
            sl = slice(lc * NB, (lc + 1) * NB)
            for ct in range(CT):
                if ct == 1:
                    nc.scalar.activation(out=h8[:, lc, ct, :],
                                         in_=x8_t[ct][:, sl],
                                         func=AF.Identity, bias=t_t[ct][:],
                                         scale=s_t[ct][:])
                else:
                    nc.vector.tensor_scalar(out=h8[:, lc, ct, :],
                                            in0=x8_t[ct][:, sl],
                                            scalar1=s_t[ct][:],
                                            scalar2=t_t[ct][:],
                                            op0=AluOpType.mult,
                                            op1=AluOpType.add)

        # ---- 1x1 convs: Q [128, ib, ci, i]; K [128, jt, ci, j] fp8 ----
        q8 = q_p.tile([128, LB, CT, NB], F8, name=nm("q8"), tag="q8")
        k8 = k_p.tile([128, JT, CT, 128], F8, name=nm("k8"), tag="k8")
        conv(bq_t, q8, load_w8(wq_d))
        conv(bk_t, k8, load_w8(wk_d), jt_layout=True)

        # ---- W~^T = (Wo Wv H)^T directly from H: the V conv is folded into
        # W' = Wo@Wv on the host; lhsT = H slices give the transposed layout
        # for free, and Wo@bv rides in with xt (softmax rows sum to 1) ----
        wvo8_sb = load_w8(wvo_d)
        wt8 = wt_p.tile([128, JT, C], F8, name=nm("wt8"), tag="wt8")
        for jt in range(JT):
            lc, js = jt // 4, jt % 4
            ps = ps_mm.tile([128, C], F32, name=nm("mm"), tag="mm")
            for p in range(2):
                nc.tensor.matmul(
                    ps[:],
                    h8[:, lc, 2 * p:2 * p + 2, js * 128:(js + 1) * 128],
                    wvo8_sb[:, 2 * p:2 * p + 2, :],
                    start=(p == 0), stop=(p == 1), perf_mode=DR)
            if jt % 2 == 0:
                nc.scalar.activation(out=wt8[:, jt, :], in_=ps[:],
                                     func=AF.Identity, bias=0.0, scale=1.0 / WS)
            else:
                nc.vector.tensor_scalar_mul(out=wt8[:, jt, :], in0=ps[:],
                                            scalar1=inv16[:])

        # ---- attention: blocks of 512 i columns ----
        for ib in range(LB):
            rsps = ps_s.tile([1, NB], F32, name=nm("rs"), tag="s")
            ops = [ps_mm.tile([128, C], F32, name=nm("mm"), tag="mm")
                   for _ in range(4)]
            xt_sbs = []
            for s in range(4):
                row = ib * NB + s * 128
                xt_sb = xt_p.tile([128, C], F32, name=nm("xt"), tag="xt")
                nc.sync.dma_start(out=xt_sb[:], in_=xt_d[row:row + 128, :])
                xt_sbs.append(xt_sb)
            pt8 = p_p.tile([128, JT, NB], F8, name=nm("pt8"), tag="pt8")
            for jp in range(JT // 2):
                for m in range(2):
                    jt = 2 * jp + m
                    sps = ps_s.tile([128, NB], F32, name=nm("s"), tag="s")
                    for p in range(2):
                        nc.tensor.matmul(
                            sps[:],
                            k8[:, jt, 2 * p:2 * p + 2, :],
                            q8[:, ib, 2 * p:2 * p + 2, :],
                            start=(p == 0), stop=(p == 1), perf_mode=DR)
                    nc.scalar.activation(out=pt8[:, jt, :], in_=sps[:],
                                         func=AF.Exp, scale=scale,
                                         bias=ebias[:])
                # row sums first: the block-tail normalize chain hangs off
                # this, so it should finish before the last out matmuls
                nc.tensor.matmul(rsps[:],
                                 one8_t[:, :, 0:1],
                                 pt8[:, 2 * jp:2 * jp + 2, :],
                                 start=(jp == 0), stop=(jp == JT // 2 - 1),
                                 perf_mode=DR)
                for s in range(4):
                    nc.tensor.matmul(
                        ops[s][:],
                        pt8[:, 2 * jp:2 * jp + 2, s * 128:(s + 1) * 128],
                        wt8[:, 2 * jp:2 * jp + 2, :],
                        start=(jp == 0), stop=(jp == JT // 2 - 1),
                        perf_mode=DR)
            rssb = small.tile([1, NB], F32, name=nm("rssb"), tag="rssb")
            nc.vector.tensor_copy(out=rssb[:], in_=rsps[:])
            rec4 = small.tile([128, 4], F32, name=nm("rec4"), tag="rec4")
            if ib < LB - 1:
                # mid-block: DMA scatter (PE is busy with the next block)
                rs4 = small.tile([128, 4], F32, name=nm("rs4"), tag="rs4")
                for s in range(4):
                    nc.sync.dma_start(out=rs4[:, s:s + 1],
                                      in_=rssb[0:1, s * 128:(s + 1) * 128])
                nc.vector.reciprocal(out=rec4[:], in_=rs4[:])
            else:
                # last block: K=1 transpose matmuls (PE idle, shortest chain)
                trp = ps_s.tile([128, 4], F32, name=nm("tr"), tag="s")
                for s in range(4):
                    nc.tensor.matmul(trp[:, s:s + 1],
                                     rssb[0:1, s * 128:(s + 1) * 128],
                                     onesf[:],
                                     start=True, stop=True)
                nc.vector.reciprocal(out=rec4[:], in_=trp[:])
            for s in range(4):
                rec = rec4[:, s:s + 1]
                row = ib * NB + s * 128
                o1 = io_p.tile([128, C], F32, name=nm("o1"), tag="o1")
                yt_sb = io_p.tile([128, C], F32, name=nm("yt"), tag="yt")
                # o1 on DVE; the residual add on GpSimd so the two stages of
                # consecutive s pipeline across engines
                nc.vector.tensor_scalar_mul(out=o1[:], in0=ops[s][:],
                                            scalar1=rec)
                nc.gpsimd.tensor_add(out=yt_sb[:], in0=o1[:],
                                     in1=xt_sbs[s][:])
                nc.sync.dma_start(out=yt_d[row:row + 128, :], in_=yt_sb[:])

    nc.compile()
    return nc


def get_nc():
    if "nc" not in _CACHE:
        _CACHE["nc"] = _build()
    return _CACHE["nc"]


def _w8(w):
    # [O, C] f32 -> fp8 [128, CT*C]: row c_in, col ci*C + o of 16*W^T
    import ml_dtypes
    wT = np.asarray(w, np.float32).T * WS          # [c, o]
    arr = wT.reshape(CT, 128, C).transpose(1, 0, 2).reshape(128, CT * C)
    return np.clip(arr, -240, 240).astype(ml_dtypes.float8_e4m3)


def make_in_maps(**inputs):
    import ml_dtypes
    x = np.asarray(inputs["x"], np.float32)
    bo = np.asarray(inputs["bo"], np.float32)
    xbias = (np.asarray(inputs["wo"], np.float64)
             @ np.asarray(inputs["bv"], np.float64)).astype(np.float32)
    gm = np.zeros((C, G), np.float32)
    gm[np.arange(C), np.arange(C) // GS] = 1.0
    wo_f = np.asarray(inputs["wo"], np.float64)
    shared = {
        "wq8": _w8(inputs["wq"]),
        "wk8": _w8(inputs["wk"]),
        "wvo8": _w8((wo_f @ np.asarray(inputs["wv"], np.float64))
                    .astype(np.float32)),
        "cvec": np.stack(
            [np.asarray(inputs[k], np.float32).reshape(CT, 128)
             for k in ("bq", "bk", "bv", "gn_w", "gn_b")],
            axis=-1).transpose(1, 0, 2).reshape(128, CT * 5).copy(),
        "gmil": gm.reshape(CT, 128, G).transpose(1, 0, 2).reshape(128, CT * G).copy(),
        "gmT": np.ascontiguousarray(gm.T),
        "one8": np.ones((128, 32), ml_dtypes.float8_e4m3),
    }
    in_maps = []
    for b in range(NCORES):
        m = dict(shared)
        m["x8"] = (x[b].reshape(CT, 128, L).transpose(1, 0, 2)
                   .reshape(128, CT * L).astype(ml_dtypes.float8_e4m3))
        m["xt"] = np.ascontiguousarray(x[b].T + bo[None, :] + xbias[None, :])
        in_maps.append(m)
    return in_maps


def kernel(**inputs):
    from concourse.bass_utils import run_bass_kernel_spmd

    nc = get_nc()
    in_maps = make_in_maps(**inputs)
    res = run_bass_kernel_spmd(nc, in_maps, core_ids=list(range(NCORES)))
    out = np.stack([res.results[b]["yt"].T for b in range(NCORES)])
    return np.ascontiguousarray(out, dtype=np.float32)
